# revision 2
# baseline (speedup 1.0000x reference)
"""MoE SwiGLU feed-forward (top-2, E=8) on 8 trn2 cores — exact-capacity EP.

Expert parallelism (core e = expert e). Host routes tokens (fp64 gating),
groups them per expert, and pads only to a multiple of 8 tokens. Per core:
  B: H[i, t] = silu(x W1^T) * (x W2^T)       fp16 matmuls, fp32 PSUM
  C: out[d, t] = gate[t] * sum_i H[i, t] W3[d, i]
Host scatter-adds the two expert contributions per token.

vs the x2-folded baseline (~358 us):
  - No x2 stream: MM2 reuses xT as the moving operand; the per-token gate
    weight is applied in the phase-C eviction as a broadcast-row multiply
    on the Vector engine. Halves the head DMA demand and frees 4.25 MB SBUF.
  - Dependency-free PE warmup matmuls (read uninitialized SBUF, result
    discarded) so the PE ramps HAM from t~0 while the head DMAs land.
  - Head DMAs in consumption order: first weight pair split per kd-half and
    interleaved with the lead x chunk on sync; pairs 1-2 kicked from the
    scalar engine in parallel.
  - First pass 2-wide i-tile interleave (x demand 147 GB/s), then i-major.
  - Phase C: evictions on vector (gate multiply), out-DMA kicks on scalar;
    last chunk split in two halves so the final copy+DMA tail is short.

Hardcoded: x [4,2048,1024], Wg [8,1024], W1/W2 [8,2048,1024], W3 [8,1024,2048].
"""

import numpy as np

P = 128
D = 1024
I = 2048
E = 8
TOP_K = 2
N_CORES = 8
KD = D // P  # 8
KI = I // P  # 16
ND = D // P  # 8 output d-tiles

_BUILD_CACHE: dict[int, object] = {}
LAST_RESULTS = None


def _chunks_of(C, lead=None):
    sizes = []
    if lead and C > lead:
        sizes.append(lead)
        C -= lead
    sizes += [512] * (C // 512)
    if C % 512:
        sizes.append(C % 512)
    out, off = [], 0
    for s in sizes:
        out.append((off, s))
        off += s
    return out


def _build_nc(C: int):
    import concourse.bass as bass  # noqa: F401
    import concourse.mybir as mybir
    import concourse.tile as tile
    from concourse import bacc

    fp16 = mybir.dt.float16
    fp32 = mybir.dt.float32
    SILU = mybir.ActivationFunctionType.Silu

    nc = bacc.Bacc(
        "TRN2",
        target_bir_lowering=False,
        debug=False,
        enable_asserts=False,
        num_devices=N_CORES,
    )

    # DRAM I/O (host-pre-tiled, contiguous per partition):
    #   xT [P, KD, C]      xT[p, kd, t] = x[t, kd*P+p]
    #   w12t [KI, P, 2, KD, P]  [it][p, s, kd, c] = W{s+1}[it*P+c, kd*P+p]
    #     (W1 and W2 interleaved so one kick fetches a full pair)
    #   w3t [P, KI, D]     w3t[p, ki, d] = W3[d, ki*P + p]
    #   gt  [P, C]         gate weight per token, replicated on partitions
    #   out [D, C] fp16    out[d, t] (host transposes back)
    xT = nc.dram_tensor("xT", [P, KD, C], fp16, kind="ExternalInput")
    w12t = nc.dram_tensor("w12t", [KI, P, 2, KD, P], fp16, kind="ExternalInput")
    w3t = nc.dram_tensor("w3t", [P, KI, D], fp16, kind="ExternalInput")
    gt = nc.dram_tensor("gt", [P, C], fp16, kind="ExternalInput")
    out = nc.dram_tensor("out", [D, C], fp16, kind="ExternalOutput")

    chunks = _chunks_of(C, lead=256)
    # Phase C: split the last chunk so the final eviction+DMA tail is short.
    t0L, twL = chunks[-1]
    if twL > 192:
        h = ((twL // 2) + 7) // 8 * 8
        cchunks = chunks[:-1] + [(t0L, h), (t0L + h, twL - h)]
    else:
        cchunks = list(chunks)

    with tile.TileContext(nc) as tc:
        with (
            tc.tile_pool(name="resident", bufs=1) as res,
            tc.tile_pool(name="wstream", bufs=6) as wpool,
            tc.tile_pool(name="tmp", bufs=4) as tmp,
            tc.tile_pool(name="outp", bufs=6) as outp,
            tc.tile_pool(name="ps1", bufs=2, space="PSUM") as ps1,
            tc.tile_pool(name="ps2", bufs=2, space="PSUM") as ps2,
            tc.tile_pool(name="ps3", bufs=4, space="PSUM") as ps3,
        ):
            xT_s = res.tile([P, KD, C], fp16)
            H = res.tile([P, KI, C], fp16)
            w3_s = res.tile([P, KI, D], fp16)
            g_s = res.tile([P, C], fp16)

            # PE p-state warmup: matmuls on just-memset SBUF (results
            # discarded) so the PE ramps HAM while the head DMAs land.
            warm_a = res.tile([P, P], fp16)
            warm_b = res.tile([P, 512], fp16)
            nc.vector.memset(warm_a[:], 0.0)
            nc.gpsimd.memset(warm_b[:], 0.0)
            # 16 cold matmuls x 427ns = 6.8us of continuous PE activity —
            # guaranteed to cover a full free-running 3.4us HAM window at any
            # phase, so the clock flips to 2.4GHz before the real stream.
            wps = ps3.tile([P, 512], fp32, tag="po")
            for _ in range(16):
                nc.tensor.matmul(
                    wps[:], warm_a[:], warm_b[:], start=True, stop=True
                )
            act_warm = tmp.tile([P, 1], fp16, tag="actw")
            nc.scalar.activation(act_warm[:], warm_a[:, :1], SILU)

            w_tiles = {}

            def alloc_w(it):
                w = wpool.tile([P, 2, KD, P], fp16, tag="w12")
                w_tiles[it] = (w[:, 0], w[:, 1])
                return w

            # Head DMAs: the critical set (w1[0], x lead chunk, w2[0]) kicks
            # from three engines in parallel so all three transfers are in
            # flight immediately (aggregate HBM rate ramps with in-flight
            # count). Everything else follows on sync in deadline order;
            # w3/gate last — phase C only.
            t0, tw = chunks[0]
            p0 = alloc_w(0)
            nc.sync.dma_start(w_tiles[0][0][:], w12t[0, :, 0])
            nc.scalar.dma_start(xT_s[:, :, t0 : t0 + tw], xT[:, :, t0 : t0 + tw])
            nc.sync.dma_start(w_tiles[0][1][:], w12t[0, :, 1])
            p1 = alloc_w(1)
            nc.sync.dma_start(p1[:], w12t[1])
            for tc0, tcw in chunks[1:]:
                nc.sync.dma_start(
                    xT_s[:, :, tc0 : tc0 + tcw], xT[:, :, tc0 : tc0 + tcw]
                )
            for it in range(2, KI):
                w = alloc_w(it)
                nc.sync.dma_start(w[:], w12t[it])
            nc.sync.dma_start(w3_s[:], w3t[:])
            nc.sync.dma_start(g_s[:], gt[:])

            # Phase B: first two i-tiles interleave chunk-by-chunk (keeps the
            # PE's fresh-byte demand under the DMA rate at the head), then
            # i-tile major.
            sched = []
            for c in chunks:
                for it in (0, 1):
                    sched.append((it, c))
            for it in range(2, KI):
                for c in chunks:
                    sched.append((it, c))

            for it, (c0, cw) in sched:
                w1_s, w2_s = w_tiles[it]
                p1 = ps1.tile([P, 512], fp32)
                p2 = ps2.tile([P, 512], fp32)
                for kd in range(KD):
                    nc.tensor.matmul(
                        p1[:, :cw],
                        w1_s[:, kd, :],
                        xT_s[:, kd, c0 : c0 + cw],
                        start=(kd == 0),
                        stop=(kd == KD - 1),
                    )
                for kd in range(KD):
                    nc.tensor.matmul(
                        p2[:, :cw],
                        w2_s[:, kd, :],
                        xT_s[:, kd, c0 : c0 + cw],
                        start=(kd == 0),
                        stop=(kd == KD - 1),
                    )
                sil = tmp.tile([P, 512], fp16)
                nc.scalar.activation(sil[:, :cw], p1[:, :cw], SILU)
                nc.vector.tensor_mul(
                    H[:, it, c0 : c0 + cw], sil[:, :cw], p2[:, :cw]
                )

            # Phase C: out[d, t] = gate[t] * sum_i H[i, t] W3[d, i] — W3 tile
            # stationary, H moving. Eviction applies the gate (vector,
            # broadcast row); out-DMA kicks go on the scalar engine.
            for c0, cw in cchunks:
                for dt in range(ND):
                    po = ps3.tile([P, 512], fp32, tag="po")
                    dsl = slice(dt * P, (dt + 1) * P)
                    for ki in range(KI):
                        nc.tensor.matmul(
                            po[:, :cw],
                            w3_s[:, ki, dsl],
                            H[:, ki, c0 : c0 + cw],
                            start=(ki == 0),
                            stop=(ki == KI - 1),
                        )
                    ot = outp.tile([P, 512], fp16)
                    nc.vector.tensor_mul(
                        ot[:, :cw], po[:, :cw], g_s[:, c0 : c0 + cw]
                    )
                    nc.scalar.dma_start(out[dsl, c0 : c0 + cw], ot[:, :cw])

    nc.compile()
    return nc


def _route(xf64: np.ndarray, Wg64: np.ndarray):
    """Top-2 routing in fp64 (selection matches jax fp32 on this dataset)."""
    scores = xf64 @ Wg64.T
    order = np.argsort(-scores, axis=1, kind="stable")[:, :TOP_K]
    s1 = np.take_along_axis(scores, order, axis=1)
    e2 = np.exp(s1[:, 1] - s1[:, 0])
    p1 = 1.0 / (1.0 + e2)
    pw = np.stack([p1, 1.0 - p1], axis=1)
    idx_list, w_list = [], []
    for e in range(E):
        mask = order == e
        tok = np.nonzero(mask.any(axis=1))[0]
        wv = (pw * mask)[tok].sum(axis=1)
        idx_list.append(tok)
        w_list.append(wv.astype(np.float32))
    return idx_list, w_list


def kernel(x, Wg, W1, W2, W3):
    global LAST_RESULTS
    from concourse.bass_utils import run_bass_kernel_spmd

    x = np.asarray(x, dtype=np.float32)
    Wg = np.asarray(Wg, dtype=np.float32)
    W1 = np.asarray(W1, dtype=np.float32)
    W2 = np.asarray(W2, dtype=np.float32)
    W3 = np.asarray(W3, dtype=np.float32)

    B, S, _ = x.shape
    T = B * S
    xf = x.reshape(T, D)

    idx_list, w_list = _route(xf.astype(np.float64), Wg.astype(np.float64))
    C = max(len(t) for t in idx_list)
    C = ((C + 7) // 8) * 8

    if C not in _BUILD_CACHE:
        _BUILD_CACHE[C] = _build_nc(C)
    nc = _BUILD_CACHE[C]

    in_maps = []
    for e in range(E):
        tok, wv = idx_list[e], w_list[e]
        n = len(tok)

        xe = np.zeros((C, D), dtype=np.float16)
        xe[:n] = xf[tok]
        xTP = np.ascontiguousarray(xe.T.reshape(KD, P, C).transpose(1, 0, 2))

        gate = np.zeros((C,), dtype=np.float16)
        gate[:n] = wv
        g2 = np.ascontiguousarray(np.broadcast_to(gate[None, :], (P, C)))

        w12P = np.empty((KI, P, 2, KD, P), dtype=np.float16)
        w12P[:, :, 0] = W1[e].reshape(KI, P, KD, P).transpose(0, 3, 2, 1)
        w12P[:, :, 1] = W2[e].reshape(KI, P, KD, P).transpose(0, 3, 2, 1)
        w3P = np.ascontiguousarray(
            W3[e].reshape(D, KI, P).transpose(2, 1, 0).astype(np.float16)
        )

        in_maps.append({"xT": xTP, "w12t": w12P, "w3t": w3P, "gt": g2})

    LAST_RESULTS = run_bass_kernel_spmd(nc, in_maps, core_ids=list(range(N_CORES)))

    outf = np.zeros((T, D), dtype=np.float32)
    for e in range(E):
        y = LAST_RESULTS.results[e]["out"]  # [D, C] fp16
        n = len(idx_list[e])
        outf[idx_list[e]] += y[:, :n].T.astype(np.float32)
    return outf.reshape(B, S, D)


# revision 3
# speedup vs baseline: 1.0794x; 1.0794x over previous
"""MoE SwiGLU feed-forward (top-2, E=8) on 8 trn2 cores — exact-capacity EP.

Expert parallelism (core e = expert e). Host routes tokens (fp64 gating),
groups them per expert, and pads only to a multiple of 8 tokens. Per core:
  B: H[i, t] = silu(x W1^T) * (x W2^T)       fp16 matmuls, fp32 PSUM
  C: out[d, t] = gate[t] * sum_i H[i, t] W3[d, i]
Host scatter-adds the two expert contributions per token.

vs the x2-folded baseline (~358 us -> ~356.4 us):
  - No x2 stream: MM2 reuses xT as the moving operand; the per-token gate
    weight is applied in the phase-C eviction as a broadcast-row multiply
    on the Vector engine. Halves the head DMA demand and frees 4.25 MB SBUF.
  - 16 warmup matmuls (6.8 us of continuous PE activity) cover a full
    free-running 3.4 us HAM window at any phase, so the PE clock is at
    2.4 GHz before the real stream and never ramps mid-kernel.
  - Head DMAs: the critical set (w1[0] on sync, x lead chunk on scalar,
    w2[0] on sync) kicks from two engines in parallel — aggregate HBM rate
    ramps with in-flight transfer count — then deadline-ordered kicks on
    sync only; w3/gate last (phase C consumes them ~220 us in).
  - First pass 2-wide i-tile interleave (x demand 147 GB/s), then i-major.
  - Phase C: evictions on vector (gate multiply), out-DMA kicks on scalar;
    last chunk split in two halves so the final copy+DMA tail is short.
Remaining time is floor: 332.8 us fp16 PE roofline (C=2080 tokens x 384
cycles/token @ 2.4 GHz) + 4.8 us NX issue (2.5 ns/MM) + ~10 us head DMA
fill + ~8.8 us fixed NEFF semaphore-teardown epilogue + ~1.5 us preamble.

Hardcoded: x [4,2048,1024], Wg [8,1024], W1/W2 [8,2048,1024], W3 [8,1024,2048].
"""

import numpy as np

P = 128
D = 1024
I = 2048
E = 8
TOP_K = 2
N_CORES = 8
KD = D // P  # 8
KI = I // P  # 16
ND = D // P  # 8 output d-tiles

_BUILD_CACHE: dict[int, object] = {}
LAST_RESULTS = None


def _chunks_of(C, lead=None):
    sizes = []
    if lead and C > lead:
        sizes.append(lead)
        C -= lead
    sizes += [512] * (C // 512)
    if C % 512:
        sizes.append(C % 512)
    out, off = [], 0
    for s in sizes:
        out.append((off, s))
        off += s
    return out


def _build_nc(C: int):
    import concourse.bass as bass  # noqa: F401
    import concourse.mybir as mybir
    import concourse.tile as tile
    from concourse import bacc

    fp16 = mybir.dt.float16
    fp32 = mybir.dt.float32
    SILU = mybir.ActivationFunctionType.Silu

    nc = bacc.Bacc(
        "TRN2",
        target_bir_lowering=False,
        debug=False,
        enable_asserts=False,
        num_devices=N_CORES,
    )

    # DRAM I/O (host-pre-tiled, contiguous per partition):
    #   xT [P, KD, C]      xT[p, kd, t] = x[t, kd*P+p]
    #   w12t [KI, P, 2, KD, P]  [it][p, s, kd, c] = W{s+1}[it*P+c, kd*P+p]
    #     (W1 and W2 interleaved so one kick fetches a full pair)
    #   w3t [P, KI, D]     w3t[p, ki, d] = W3[d, ki*P + p]
    #   gt  [P, C]         gate weight per token, replicated on partitions
    #   out [D, C] fp16    out[d, t] (host transposes back)
    xT = nc.dram_tensor("xT", [P, KD, C], fp16, kind="ExternalInput")
    w12t = nc.dram_tensor("w12t", [KI, P, 2, KD, P], fp16, kind="ExternalInput")
    w3t = nc.dram_tensor("w3t", [P, KI, D], fp16, kind="ExternalInput")
    gt = nc.dram_tensor("gt", [P, C], fp16, kind="ExternalInput")
    out = nc.dram_tensor("out", [D, C], fp16, kind="ExternalOutput")

    chunks = _chunks_of(C, lead=256)
    # Phase C: split the last chunk so the final eviction+DMA tail is short.
    t0L, twL = chunks[-1]
    if twL > 192:
        h = ((twL // 2) + 7) // 8 * 8
        cchunks = chunks[:-1] + [(t0L, h), (t0L + h, twL - h)]
    else:
        cchunks = list(chunks)

    with tile.TileContext(nc) as tc:
        with (
            tc.tile_pool(name="resident", bufs=1) as res,
            tc.tile_pool(name="wstream", bufs=6) as wpool,
            tc.tile_pool(name="tmp", bufs=4) as tmp,
            tc.tile_pool(name="outp", bufs=6) as outp,
            tc.tile_pool(name="ps1", bufs=2, space="PSUM") as ps1,
            tc.tile_pool(name="ps2", bufs=2, space="PSUM") as ps2,
            tc.tile_pool(name="ps3", bufs=4, space="PSUM") as ps3,
        ):
            xT_s = res.tile([P, KD, C], fp16)
            H = res.tile([P, KI, C], fp16)
            w3_s = res.tile([P, KI, D], fp16)
            g_s = res.tile([P, C], fp16)

            # PE p-state warmup: matmuls on just-memset SBUF (results
            # discarded) so the PE ramps HAM while the head DMAs land.
            warm_a = res.tile([P, P], fp16)
            warm_b = res.tile([P, 512], fp16)
            nc.vector.memset(warm_a[:], 0.0)
            nc.gpsimd.memset(warm_b[:], 0.0)
            # 16 cold matmuls x 427ns = 6.8us of continuous PE activity —
            # guaranteed to cover a full free-running 3.4us HAM window at any
            # phase, so the clock flips to 2.4GHz before the real stream.
            wps = ps3.tile([P, 512], fp32, tag="po")
            for _ in range(16):
                nc.tensor.matmul(
                    wps[:], warm_a[:], warm_b[:], start=True, stop=True
                )
            act_warm = tmp.tile([P, 1], fp16, tag="actw")
            nc.scalar.activation(act_warm[:], warm_a[:, :1], SILU)

            w_tiles = {}

            def alloc_w(it):
                w = wpool.tile([P, 2, KD, P], fp16, tag="w12")
                w_tiles[it] = (w[:, 0], w[:, 1])
                return w

            # Head DMAs: the critical set (w1[0], x lead chunk, w2[0]) kicks
            # from three engines in parallel so all three transfers are in
            # flight immediately (aggregate HBM rate ramps with in-flight
            # count). Everything else follows on sync in deadline order;
            # w3/gate last — phase C only.
            t0, tw = chunks[0]
            p0 = alloc_w(0)
            nc.sync.dma_start(w_tiles[0][0][:], w12t[0, :, 0])
            nc.scalar.dma_start(xT_s[:, :, t0 : t0 + tw], xT[:, :, t0 : t0 + tw])
            nc.sync.dma_start(w_tiles[0][1][:], w12t[0, :, 1])
            p1 = alloc_w(1)
            nc.sync.dma_start(p1[:], w12t[1])
            for tc0, tcw in chunks[1:]:
                nc.sync.dma_start(
                    xT_s[:, :, tc0 : tc0 + tcw], xT[:, :, tc0 : tc0 + tcw]
                )
            for it in range(2, KI):
                w = alloc_w(it)
                nc.sync.dma_start(w[:], w12t[it])
            nc.sync.dma_start(w3_s[:], w3t[:])
            nc.sync.dma_start(g_s[:], gt[:])

            # Phase B: first two i-tiles interleave chunk-by-chunk (keeps the
            # PE's fresh-byte demand under the DMA rate at the head), then
            # i-tile major.
            sched = []
            for c in chunks:
                for it in (0, 1):
                    sched.append((it, c))
            for it in range(2, KI):
                for c in chunks:
                    sched.append((it, c))

            for it, (c0, cw) in sched:
                w1_s, w2_s = w_tiles[it]
                p1 = ps1.tile([P, 512], fp32)
                p2 = ps2.tile([P, 512], fp32)
                for kd in range(KD):
                    nc.tensor.matmul(
                        p1[:, :cw],
                        w1_s[:, kd, :],
                        xT_s[:, kd, c0 : c0 + cw],
                        start=(kd == 0),
                        stop=(kd == KD - 1),
                    )
                for kd in range(KD):
                    nc.tensor.matmul(
                        p2[:, :cw],
                        w2_s[:, kd, :],
                        xT_s[:, kd, c0 : c0 + cw],
                        start=(kd == 0),
                        stop=(kd == KD - 1),
                    )
                sil = tmp.tile([P, 512], fp16)
                nc.scalar.activation(sil[:, :cw], p1[:, :cw], SILU)
                nc.vector.tensor_mul(
                    H[:, it, c0 : c0 + cw], sil[:, :cw], p2[:, :cw]
                )

            # Phase C: out[d, t] = gate[t] * sum_i H[i, t] W3[d, i] — W3 tile
            # stationary, H moving. Eviction applies the gate (vector,
            # broadcast row); out-DMA kicks go on the scalar engine.
            for c0, cw in cchunks:
                for dt in range(ND):
                    po = ps3.tile([P, 512], fp32, tag="po")
                    dsl = slice(dt * P, (dt + 1) * P)
                    for ki in range(KI):
                        nc.tensor.matmul(
                            po[:, :cw],
                            w3_s[:, ki, dsl],
                            H[:, ki, c0 : c0 + cw],
                            start=(ki == 0),
                            stop=(ki == KI - 1),
                        )
                    ot = outp.tile([P, 512], fp16)
                    nc.vector.tensor_mul(
                        ot[:, :cw], po[:, :cw], g_s[:, c0 : c0 + cw]
                    )
                    nc.scalar.dma_start(out[dsl, c0 : c0 + cw], ot[:, :cw])

    nc.compile()
    return nc


def _route(xf64: np.ndarray, Wg64: np.ndarray):
    """Top-2 routing in fp64 (selection matches jax fp32 on this dataset)."""
    scores = xf64 @ Wg64.T
    order = np.argsort(-scores, axis=1, kind="stable")[:, :TOP_K]
    s1 = np.take_along_axis(scores, order, axis=1)
    e2 = np.exp(s1[:, 1] - s1[:, 0])
    p1 = 1.0 / (1.0 + e2)
    pw = np.stack([p1, 1.0 - p1], axis=1)
    idx_list, w_list = [], []
    for e in range(E):
        mask = order == e
        tok = np.nonzero(mask.any(axis=1))[0]
        wv = (pw * mask)[tok].sum(axis=1)
        idx_list.append(tok)
        w_list.append(wv.astype(np.float32))
    return idx_list, w_list


def kernel(x, Wg, W1, W2, W3):
    global LAST_RESULTS
    from concourse.bass_utils import run_bass_kernel_spmd

    x = np.asarray(x, dtype=np.float32)
    Wg = np.asarray(Wg, dtype=np.float32)
    W1 = np.asarray(W1, dtype=np.float32)
    W2 = np.asarray(W2, dtype=np.float32)
    W3 = np.asarray(W3, dtype=np.float32)

    B, S, _ = x.shape
    T = B * S
    xf = x.reshape(T, D)

    idx_list, w_list = _route(xf.astype(np.float64), Wg.astype(np.float64))
    C = max(len(t) for t in idx_list)
    C = ((C + 7) // 8) * 8

    if C not in _BUILD_CACHE:
        _BUILD_CACHE[C] = _build_nc(C)
    nc = _BUILD_CACHE[C]

    in_maps = []
    for e in range(E):
        tok, wv = idx_list[e], w_list[e]
        n = len(tok)

        xe = np.zeros((C, D), dtype=np.float16)
        xe[:n] = xf[tok]
        xTP = np.ascontiguousarray(xe.T.reshape(KD, P, C).transpose(1, 0, 2))

        gate = np.zeros((C,), dtype=np.float16)
        gate[:n] = wv
        g2 = np.ascontiguousarray(np.broadcast_to(gate[None, :], (P, C)))

        w12P = np.empty((KI, P, 2, KD, P), dtype=np.float16)
        w12P[:, :, 0] = W1[e].reshape(KI, P, KD, P).transpose(0, 3, 2, 1)
        w12P[:, :, 1] = W2[e].reshape(KI, P, KD, P).transpose(0, 3, 2, 1)
        w3P = np.ascontiguousarray(
            W3[e].reshape(D, KI, P).transpose(2, 1, 0).astype(np.float16)
        )

        in_maps.append({"xT": xTP, "w12t": w12P, "w3t": w3P, "gt": g2})

    LAST_RESULTS = run_bass_kernel_spmd(nc, in_maps, core_ids=list(range(N_CORES)))

    outf = np.zeros((T, D), dtype=np.float32)
    for e in range(E):
        y = LAST_RESULTS.results[e]["out"]  # [D, C] fp16
        n = len(idx_list[e])
        outf[idx_list[e]] += y[:, :n].T.astype(np.float32)
    return outf.reshape(B, S, D)


# revision 6
# speedup vs baseline: 1.0802x; 1.0007x over previous
"""MoE SwiGLU feed-forward (top-2, E=8) on 8 trn2 cores — exact-capacity EP.

Expert parallelism (core e = expert e). Host routes tokens (fp64 gating),
groups them per expert, and pads only to a multiple of 2 tokens. Per core:
  B: H[i, t] = silu(x W1^T) * (x W2^T)       fp16 matmuls, fp32 PSUM
  C: out[d, t] = gate[t] * sum_i H[i, t] W3[d, i]
Host scatter-adds the two expert contributions per token.

vs the x2-folded baseline (~358 us):
  - No x2 stream: MM2 reuses xT as the moving operand; the per-token gate
    weight is applied in the phase-C eviction as a broadcast-row multiply
    on the Vector engine. Halves the head DMA demand and frees 4.25 MB SBUF.
  - Dependency-free PE warmup matmuls (read uninitialized SBUF, result
    discarded) so the PE ramps HAM from t~0 while the head DMAs land.
  - Head DMAs in consumption order: first weight pair split per kd-half and
    interleaved with the lead x chunk on sync; pairs 1-2 kicked from the
    scalar engine in parallel.
  - First pass 2-wide i-tile interleave (x demand 147 GB/s), then i-major.
  - Phase C: evictions on vector (gate multiply), out-DMA kicks on scalar;
    last chunk split in two halves so the final copy+DMA tail is short.

Hardcoded: x [4,2048,1024], Wg [8,1024], W1/W2 [8,2048,1024], W3 [8,1024,2048].
"""

import numpy as np

P = 128
D = 1024
I = 2048
E = 8
TOP_K = 2
N_CORES = 8
KD = D // P  # 8
KI = I // P  # 16
ND = D // P  # 8 output d-tiles

_BUILD_CACHE: dict[int, object] = {}
LAST_RESULTS = None


def _chunks_of(C, lead=None):
    """Geometric head (256, 384) then 512s: smaller early chunks smooth the
    head DMA supply-demand curve so the 2-wide first pass never outruns HBM.
    All chunks >= 256 so LDWEIGHTS stays hidden under the matmul stream."""
    sizes = []
    for s in (256, 384):
        if C - sum(sizes) >= s + 256:
            sizes.append(s)
    while C - sum(sizes) >= 512 + 256:
        sizes.append(512)
    rem = C - sum(sizes)
    if rem > 512:
        sizes += [rem - 256, 256]
    elif rem:
        sizes.append(rem)
    out, off = [], 0
    for s in sizes:
        out.append((off, s))
        off += s
    return out


def _build_nc(C: int):
    import concourse.bass as bass  # noqa: F401
    import concourse.mybir as mybir
    import concourse.tile as tile
    from concourse import bacc

    fp16 = mybir.dt.float16
    fp32 = mybir.dt.float32
    SILU = mybir.ActivationFunctionType.Silu

    nc = bacc.Bacc(
        "TRN2",
        target_bir_lowering=False,
        debug=False,
        enable_asserts=False,
        num_devices=N_CORES,
    )

    # DRAM I/O (host-pre-tiled, contiguous per partition):
    #   xT [P, KD, C]      xT[p, kd, t] = x[t, kd*P+p]
    #   w12t [KI, P, 2, KD, P]  [it][p, s, kd, c] = W{s+1}[it*P+c, kd*P+p]
    #     (W1 and W2 interleaved so one kick fetches a full pair)
    #   w3t [P, KI, D]     w3t[p, ki, d] = W3[d, ki*P + p]
    #   gt  [P, C]         gate weight per token, replicated on partitions
    #   out [D, C] fp16    out[d, t] (host transposes back)
    xT = nc.dram_tensor("xT", [P, KD, C], fp16, kind="ExternalInput")
    w12t = nc.dram_tensor("w12t", [KI, P, 2, KD, P], fp16, kind="ExternalInput")
    w3t = nc.dram_tensor("w3t", [P, KI, D], fp16, kind="ExternalInput")
    gt = nc.dram_tensor("gt", [P, C], fp16, kind="ExternalInput")
    out = nc.dram_tensor("out", [D, C], fp16, kind="ExternalOutput")

    chunks = _chunks_of(C, lead=256)
    # Phase C: split the last chunk so the final eviction+DMA tail is short.
    t0L, twL = chunks[-1]
    if twL > 192:
        h = ((twL // 2) + 7) // 8 * 8
        cchunks = chunks[:-1] + [(t0L, h), (t0L + h, twL - h)]
    else:
        cchunks = list(chunks)

    with tile.TileContext(nc) as tc:
        with (
            tc.tile_pool(name="resident", bufs=1) as res,
            tc.tile_pool(name="wstream", bufs=6) as wpool,
            tc.tile_pool(name="tmp", bufs=4) as tmp,
            tc.tile_pool(name="outp", bufs=6) as outp,
            tc.tile_pool(name="ps1", bufs=2, space="PSUM") as ps1,
            tc.tile_pool(name="ps2", bufs=2, space="PSUM") as ps2,
            tc.tile_pool(name="ps3", bufs=4, space="PSUM") as ps3,
        ):
            xT_s = res.tile([P, KD, C], fp16)
            H = res.tile([P, KI, C], fp16)
            w3_s = res.tile([P, KI, D], fp16)
            g_s = res.tile([P, C], fp16)

            # PE p-state warmup: matmuls on just-memset SBUF (results
            # discarded) so the PE ramps HAM while the head DMAs land.
            warm_a = res.tile([P, P], fp16)
            warm_b = res.tile([P, 512], fp16)
            nc.vector.memset(warm_a[:], 0.0)
            nc.gpsimd.memset(warm_b[:], 0.0)
            # 16 cold matmuls x 427ns = 6.8us of continuous PE activity —
            # guaranteed to cover a full free-running 3.4us HAM window at any
            # phase, so the clock flips to 2.4GHz before the real stream.
            wps = ps3.tile([P, 512], fp32, tag="po")
            for _ in range(16):
                nc.tensor.matmul(
                    wps[:], warm_a[:], warm_b[:], start=True, stop=True
                )
            act_warm = tmp.tile([P, 1], fp16, tag="actw")
            nc.scalar.activation(act_warm[:], warm_a[:, :1], SILU)

            w_tiles = {}

            def alloc_w(it):
                w = wpool.tile([P, 2, KD, P], fp16, tag="w12")
                w_tiles[it] = (w[:, 0], w[:, 1])
                return w

            # Head DMAs: the critical set (w1[0], x lead chunk, w2[0]) kicks
            # from three engines in parallel so all three transfers are in
            # flight immediately (aggregate HBM rate ramps with in-flight
            # count). Everything else follows on sync in deadline order;
            # w3/gate last — phase C only.
            t0, tw = chunks[0]
            p0 = alloc_w(0)
            nc.sync.dma_start(w_tiles[0][0][:], w12t[0, :, 0])
            nc.scalar.dma_start(xT_s[:, :, t0 : t0 + tw], xT[:, :, t0 : t0 + tw])
            nc.sync.dma_start(w_tiles[0][1][:], w12t[0, :, 1])
            p1 = alloc_w(1)
            nc.sync.dma_start(p1[:], w12t[1])
            for tc0, tcw in chunks[1:]:
                nc.sync.dma_start(
                    xT_s[:, :, tc0 : tc0 + tcw], xT[:, :, tc0 : tc0 + tcw]
                )
            for it in range(2, KI):
                w = alloc_w(it)
                nc.sync.dma_start(w[:], w12t[it])
            nc.sync.dma_start(w3_s[:], w3t[:])
            nc.sync.dma_start(g_s[:], gt[:])

            # Phase B: first two i-tiles interleave chunk-by-chunk (keeps the
            # PE's fresh-byte demand under the DMA rate at the head), then
            # i-tile major.
            sched = []
            for c in chunks:
                for it in (0, 1):
                    sched.append((it, c))
            for it in range(2, KI):
                for c in chunks:
                    sched.append((it, c))

            for it, (c0, cw) in sched:
                w1_s, w2_s = w_tiles[it]
                p1 = ps1.tile([P, 512], fp32)
                p2 = ps2.tile([P, 512], fp32)
                for kd in range(KD):
                    nc.tensor.matmul(
                        p1[:, :cw],
                        w1_s[:, kd, :],
                        xT_s[:, kd, c0 : c0 + cw],
                        start=(kd == 0),
                        stop=(kd == KD - 1),
                    )
                for kd in range(KD):
                    nc.tensor.matmul(
                        p2[:, :cw],
                        w2_s[:, kd, :],
                        xT_s[:, kd, c0 : c0 + cw],
                        start=(kd == 0),
                        stop=(kd == KD - 1),
                    )
                sil = tmp.tile([P, 512], fp16)
                nc.scalar.activation(sil[:, :cw], p1[:, :cw], SILU)
                nc.vector.tensor_mul(
                    H[:, it, c0 : c0 + cw], sil[:, :cw], p2[:, :cw]
                )

            # Phase C: out[d, t] = gate[t] * sum_i H[i, t] W3[d, i] — W3 tile
            # stationary, H moving. Eviction applies the gate (vector,
            # broadcast row); out-DMA kicks go on the scalar engine.
            for c0, cw in cchunks:
                for dt in range(ND):
                    po = ps3.tile([P, 512], fp32, tag="po")
                    dsl = slice(dt * P, (dt + 1) * P)
                    for ki in range(KI):
                        nc.tensor.matmul(
                            po[:, :cw],
                            w3_s[:, ki, dsl],
                            H[:, ki, c0 : c0 + cw],
                            start=(ki == 0),
                            stop=(ki == KI - 1),
                        )
                    ot = outp.tile([P, 512], fp16)
                    nc.vector.tensor_mul(
                        ot[:, :cw], po[:, :cw], g_s[:, c0 : c0 + cw]
                    )
                    nc.scalar.dma_start(out[dsl, c0 : c0 + cw], ot[:, :cw])

    nc.compile()
    return nc


def _route(xf64: np.ndarray, Wg64: np.ndarray):
    """Top-2 routing in fp64 (selection matches jax fp32 on this dataset)."""
    scores = xf64 @ Wg64.T
    order = np.argsort(-scores, axis=1, kind="stable")[:, :TOP_K]
    s1 = np.take_along_axis(scores, order, axis=1)
    e2 = np.exp(s1[:, 1] - s1[:, 0])
    p1 = 1.0 / (1.0 + e2)
    pw = np.stack([p1, 1.0 - p1], axis=1)
    idx_list, w_list = [], []
    for e in range(E):
        mask = order == e
        tok = np.nonzero(mask.any(axis=1))[0]
        wv = (pw * mask)[tok].sum(axis=1)
        idx_list.append(tok)
        w_list.append(wv.astype(np.float32))
    return idx_list, w_list


def kernel(x, Wg, W1, W2, W3):
    global LAST_RESULTS
    from concourse.bass_utils import run_bass_kernel_spmd

    x = np.asarray(x, dtype=np.float32)
    Wg = np.asarray(Wg, dtype=np.float32)
    W1 = np.asarray(W1, dtype=np.float32)
    W2 = np.asarray(W2, dtype=np.float32)
    W3 = np.asarray(W3, dtype=np.float32)

    B, S, _ = x.shape
    T = B * S
    xf = x.reshape(T, D)

    idx_list, w_list = _route(xf.astype(np.float64), Wg.astype(np.float64))
    C = max(len(t) for t in idx_list)
    C = ((C + 1) // 2) * 2

    if C not in _BUILD_CACHE:
        _BUILD_CACHE[C] = _build_nc(C)
    nc = _BUILD_CACHE[C]

    in_maps = []
    for e in range(E):
        tok, wv = idx_list[e], w_list[e]
        n = len(tok)

        xe = np.zeros((C, D), dtype=np.float16)
        xe[:n] = xf[tok]
        xTP = np.ascontiguousarray(xe.T.reshape(KD, P, C).transpose(1, 0, 2))

        gate = np.zeros((C,), dtype=np.float16)
        gate[:n] = wv
        g2 = np.ascontiguousarray(np.broadcast_to(gate[None, :], (P, C)))

        w12P = np.empty((KI, P, 2, KD, P), dtype=np.float16)
        w12P[:, :, 0] = W1[e].reshape(KI, P, KD, P).transpose(0, 3, 2, 1)
        w12P[:, :, 1] = W2[e].reshape(KI, P, KD, P).transpose(0, 3, 2, 1)
        w3P = np.ascontiguousarray(
            W3[e].reshape(D, KI, P).transpose(2, 1, 0).astype(np.float16)
        )

        in_maps.append({"xT": xTP, "w12t": w12P, "w3t": w3P, "gt": g2})

    LAST_RESULTS = run_bass_kernel_spmd(nc, in_maps, core_ids=list(range(N_CORES)))

    outf = np.zeros((T, D), dtype=np.float32)
    for e in range(E):
        y = LAST_RESULTS.results[e]["out"]  # [D, C] fp16
        n = len(idx_list[e])
        outf[idx_list[e]] += y[:, :n].T.astype(np.float32)
    return outf.reshape(B, S, D)


# revision 7
# speedup vs baseline: 1.0830x; 1.0026x over previous
"""MoE SwiGLU feed-forward (top-2, E=8) on 8 trn2 cores — exact-capacity EP.

Expert parallelism (core e = expert e). Host routes tokens (fp64 gating),
groups them per expert, and pads only to a multiple of 2 tokens. Per core:
  B: H[i, t] = silu(x W1^T) * (x W2^T)       fp16 matmuls, fp32 PSUM
  C: out[d, t] = gate[t] * sum_i H[i, t] W3[d, i]
Host scatter-adds the two expert contributions per token.

vs the x2-folded baseline (~358 us -> ~356.2 us):
  - No x2 stream: MM2 reuses xT as the moving operand; the per-token gate
    weight is applied in the phase-C eviction as a broadcast-row multiply
    on the Vector engine. Halves the head DMA demand and frees 4.25 MB SBUF.
  - 16 warmup matmuls on memset tiles (6.8 us of continuous PE activity)
    cover a full free-running 3.4 us HAM window at any phase, so the PE
    clock is at 2.4 GHz before the real stream and never ramps mid-kernel.
    (Do NOT split this block into the real schedule — produces NaN.)
  - Head DMAs: the critical set (w1[0] on sync, x lead chunk on scalar,
    w2[0] on sync) kicks from two engines in parallel — aggregate HBM rate
    ramps with in-flight transfer count — then deadline-ordered kicks on
    sync only; w3/gate last (phase C consumes them ~220 us in).
  - Geometric head chunks (256, 384, then 512s) + 2-wide first i-tile pass:
    head supply-demand curves cross with zero PE stall at nominal DMA rate
    (measured zero gaps >230 ns over the whole stream on good runs).
  - C pads to a multiple of 2 (2078), not 8 — capacity is the max expert
    load, and every core pays C x 384 PE cycles under SPMD.
  - Phase C: evictions on vector (gate multiply), out-DMA kicks on scalar;
    last chunk split in two halves so the final copy+DMA tail is short.
Remaining time is floor: 332.5 us fp16 PE roofline (C=2078 x 384 cycles/token
@ 2.4 GHz) + 4.8 us NX issue (2.5 ns/MM) + ~10 us head DMA fill + ~8.8 us
fixed NEFF semaphore-teardown epilogue + ~1.5 us preamble. fp8 fails the
2e-2 gate (measured: MM3-only 3.7%, all-fp8 6.5%).

Hardcoded: x [4,2048,1024], Wg [8,1024], W1/W2 [8,2048,1024], W3 [8,1024,2048].
"""

import numpy as np

P = 128
D = 1024
I = 2048
E = 8
TOP_K = 2
N_CORES = 8
KD = D // P  # 8
KI = I // P  # 16
ND = D // P  # 8 output d-tiles

_BUILD_CACHE: dict[int, object] = {}
LAST_RESULTS = None


def _chunks_of(C, lead=None):
    """Geometric head (256, 384) then 512s: smaller early chunks smooth the
    head DMA supply-demand curve so the 2-wide first pass never outruns HBM.
    All chunks >= 256 so LDWEIGHTS stays hidden under the matmul stream."""
    sizes = []
    for s in (256, 384):
        if C - sum(sizes) >= s + 256:
            sizes.append(s)
    while C - sum(sizes) >= 512 + 256:
        sizes.append(512)
    rem = C - sum(sizes)
    if rem > 512:
        sizes += [rem - 256, 256]
    elif rem:
        sizes.append(rem)
    out, off = [], 0
    for s in sizes:
        out.append((off, s))
        off += s
    return out


def _build_nc(C: int):
    import concourse.bass as bass  # noqa: F401
    import concourse.mybir as mybir
    import concourse.tile as tile
    from concourse import bacc

    fp16 = mybir.dt.float16
    fp32 = mybir.dt.float32
    SILU = mybir.ActivationFunctionType.Silu

    nc = bacc.Bacc(
        "TRN2",
        target_bir_lowering=False,
        debug=False,
        enable_asserts=False,
        num_devices=N_CORES,
    )

    # DRAM I/O (host-pre-tiled, contiguous per partition):
    #   xT [P, KD, C]      xT[p, kd, t] = x[t, kd*P+p]
    #   w12t [KI, P, 2, KD, P]  [it][p, s, kd, c] = W{s+1}[it*P+c, kd*P+p]
    #     (W1 and W2 interleaved so one kick fetches a full pair)
    #   w3t [P, KI, D]     w3t[p, ki, d] = W3[d, ki*P + p]
    #   gt  [P, C]         gate weight per token, replicated on partitions
    #   out [D, C] fp16    out[d, t] (host transposes back)
    xT = nc.dram_tensor("xT", [P, KD, C], fp16, kind="ExternalInput")
    w12t = nc.dram_tensor("w12t", [KI, P, 2, KD, P], fp16, kind="ExternalInput")
    w3t = nc.dram_tensor("w3t", [P, KI, D], fp16, kind="ExternalInput")
    gt = nc.dram_tensor("gt", [P, C], fp16, kind="ExternalInput")
    out = nc.dram_tensor("out", [D, C], fp16, kind="ExternalOutput")

    chunks = _chunks_of(C, lead=256)
    # Phase C: split the last chunk so the final eviction+DMA tail is short.
    t0L, twL = chunks[-1]
    if twL > 192:
        h = ((twL // 2) + 7) // 8 * 8
        cchunks = chunks[:-1] + [(t0L, h), (t0L + h, twL - h)]
    else:
        cchunks = list(chunks)

    with tile.TileContext(nc) as tc:
        with (
            tc.tile_pool(name="resident", bufs=1) as res,
            tc.tile_pool(name="wstream", bufs=6) as wpool,
            tc.tile_pool(name="tmp", bufs=4) as tmp,
            tc.tile_pool(name="outp", bufs=6) as outp,
            tc.tile_pool(name="ps1", bufs=2, space="PSUM") as ps1,
            tc.tile_pool(name="ps2", bufs=2, space="PSUM") as ps2,
            tc.tile_pool(name="ps3", bufs=4, space="PSUM") as ps3,
        ):
            xT_s = res.tile([P, KD, C], fp16)
            H = res.tile([P, KI, C], fp16)
            w3_s = res.tile([P, KI, D], fp16)
            g_s = res.tile([P, C], fp16)

            # PE p-state warmup: matmuls on just-memset SBUF (results
            # discarded) so the PE ramps HAM while the head DMAs land.
            warm_a = res.tile([P, P], fp16)
            warm_b = res.tile([P, 512], fp16)
            nc.vector.memset(warm_a[:], 0.0)
            nc.gpsimd.memset(warm_b[:], 0.0)
            # 16 cold matmuls x 427ns = 6.8us of continuous PE activity —
            # guaranteed to cover a full free-running 3.4us HAM window at any
            # phase, so the clock flips to 2.4GHz before the real stream.
            wps = ps3.tile([P, 512], fp32, tag="po")
            for _ in range(16):
                nc.tensor.matmul(
                    wps[:], warm_a[:], warm_b[:], start=True, stop=True
                )
            act_warm = tmp.tile([P, 1], fp16, tag="actw")
            nc.scalar.activation(act_warm[:], warm_a[:, :1], SILU)

            w_tiles = {}

            def alloc_w(it):
                w = wpool.tile([P, 2, KD, P], fp16, tag="w12")
                w_tiles[it] = (w[:, 0], w[:, 1])
                return w

            # Head DMAs: the critical set (w1[0], x lead chunk, w2[0]) kicks
            # from three engines in parallel so all three transfers are in
            # flight immediately (aggregate HBM rate ramps with in-flight
            # count). Everything else follows on sync in deadline order;
            # w3/gate last — phase C only.
            t0, tw = chunks[0]
            p0 = alloc_w(0)
            nc.sync.dma_start(w_tiles[0][0][:], w12t[0, :, 0])
            nc.scalar.dma_start(xT_s[:, :, t0 : t0 + tw], xT[:, :, t0 : t0 + tw])
            nc.sync.dma_start(w_tiles[0][1][:], w12t[0, :, 1])
            p1 = alloc_w(1)
            nc.sync.dma_start(p1[:], w12t[1])
            for tc0, tcw in chunks[1:]:
                nc.sync.dma_start(
                    xT_s[:, :, tc0 : tc0 + tcw], xT[:, :, tc0 : tc0 + tcw]
                )
            for it in range(2, KI):
                w = alloc_w(it)
                nc.sync.dma_start(w[:], w12t[it])
            nc.sync.dma_start(w3_s[:], w3t[:])
            nc.sync.dma_start(g_s[:], gt[:])

            # Phase B: first two i-tiles interleave chunk-by-chunk (keeps the
            # PE's fresh-byte demand under the DMA rate at the head), then
            # i-tile major.
            sched = []
            for c in chunks:
                for it in (0, 1):
                    sched.append((it, c))
            for it in range(2, KI):
                for c in chunks:
                    sched.append((it, c))

            for it, (c0, cw) in sched:
                w1_s, w2_s = w_tiles[it]
                p1 = ps1.tile([P, 512], fp32)
                p2 = ps2.tile([P, 512], fp32)
                for kd in range(KD):
                    nc.tensor.matmul(
                        p1[:, :cw],
                        w1_s[:, kd, :],
                        xT_s[:, kd, c0 : c0 + cw],
                        start=(kd == 0),
                        stop=(kd == KD - 1),
                    )
                for kd in range(KD):
                    nc.tensor.matmul(
                        p2[:, :cw],
                        w2_s[:, kd, :],
                        xT_s[:, kd, c0 : c0 + cw],
                        start=(kd == 0),
                        stop=(kd == KD - 1),
                    )
                sil = tmp.tile([P, 512], fp16)
                nc.scalar.activation(sil[:, :cw], p1[:, :cw], SILU)
                nc.vector.tensor_mul(
                    H[:, it, c0 : c0 + cw], sil[:, :cw], p2[:, :cw]
                )

            # Phase C: out[d, t] = gate[t] * sum_i H[i, t] W3[d, i] — W3 tile
            # stationary, H moving. Eviction applies the gate (vector,
            # broadcast row); out-DMA kicks go on the scalar engine.
            for c0, cw in cchunks:
                for dt in range(ND):
                    po = ps3.tile([P, 512], fp32, tag="po")
                    dsl = slice(dt * P, (dt + 1) * P)
                    for ki in range(KI):
                        nc.tensor.matmul(
                            po[:, :cw],
                            w3_s[:, ki, dsl],
                            H[:, ki, c0 : c0 + cw],
                            start=(ki == 0),
                            stop=(ki == KI - 1),
                        )
                    ot = outp.tile([P, 512], fp16)
                    nc.vector.tensor_mul(
                        ot[:, :cw], po[:, :cw], g_s[:, c0 : c0 + cw]
                    )
                    nc.scalar.dma_start(out[dsl, c0 : c0 + cw], ot[:, :cw])

    nc.compile()
    return nc


def _route(xf64: np.ndarray, Wg64: np.ndarray):
    """Top-2 routing in fp64 (selection matches jax fp32 on this dataset)."""
    scores = xf64 @ Wg64.T
    order = np.argsort(-scores, axis=1, kind="stable")[:, :TOP_K]
    s1 = np.take_along_axis(scores, order, axis=1)
    e2 = np.exp(s1[:, 1] - s1[:, 0])
    p1 = 1.0 / (1.0 + e2)
    pw = np.stack([p1, 1.0 - p1], axis=1)
    idx_list, w_list = [], []
    for e in range(E):
        mask = order == e
        tok = np.nonzero(mask.any(axis=1))[0]
        wv = (pw * mask)[tok].sum(axis=1)
        idx_list.append(tok)
        w_list.append(wv.astype(np.float32))
    return idx_list, w_list


def kernel(x, Wg, W1, W2, W3):
    global LAST_RESULTS
    from concourse.bass_utils import run_bass_kernel_spmd

    x = np.asarray(x, dtype=np.float32)
    Wg = np.asarray(Wg, dtype=np.float32)
    W1 = np.asarray(W1, dtype=np.float32)
    W2 = np.asarray(W2, dtype=np.float32)
    W3 = np.asarray(W3, dtype=np.float32)

    B, S, _ = x.shape
    T = B * S
    xf = x.reshape(T, D)

    idx_list, w_list = _route(xf.astype(np.float64), Wg.astype(np.float64))
    C = max(len(t) for t in idx_list)
    C = ((C + 1) // 2) * 2

    if C not in _BUILD_CACHE:
        _BUILD_CACHE[C] = _build_nc(C)
    nc = _BUILD_CACHE[C]

    in_maps = []
    for e in range(E):
        tok, wv = idx_list[e], w_list[e]
        n = len(tok)

        xe = np.zeros((C, D), dtype=np.float16)
        xe[:n] = xf[tok]
        xTP = np.ascontiguousarray(xe.T.reshape(KD, P, C).transpose(1, 0, 2))

        gate = np.zeros((C,), dtype=np.float16)
        gate[:n] = wv
        g2 = np.ascontiguousarray(np.broadcast_to(gate[None, :], (P, C)))

        w12P = np.empty((KI, P, 2, KD, P), dtype=np.float16)
        w12P[:, :, 0] = W1[e].reshape(KI, P, KD, P).transpose(0, 3, 2, 1)
        w12P[:, :, 1] = W2[e].reshape(KI, P, KD, P).transpose(0, 3, 2, 1)
        w3P = np.ascontiguousarray(
            W3[e].reshape(D, KI, P).transpose(2, 1, 0).astype(np.float16)
        )

        in_maps.append({"xT": xTP, "w12t": w12P, "w3t": w3P, "gt": g2})

    LAST_RESULTS = run_bass_kernel_spmd(nc, in_maps, core_ids=list(range(N_CORES)))

    outf = np.zeros((T, D), dtype=np.float32)
    for e in range(E):
        y = LAST_RESULTS.results[e]["out"]  # [D, C] fp16
        n = len(idx_list[e])
        outf[idx_list[e]] += y[:, :n].T.astype(np.float32)
    return outf.reshape(B, S, D)
